# revision 9
# baseline (speedup 1.0000x reference)
"""Trainium2 Bass kernel for batched windowed multi-head attention.

Shapes: x (8, 64, 256, 512) f32, H=8 heads, D=64.
Sharding: data-parallel over batch dim B=8 -> 1 batch row per NeuronCore.
Each core processes 64 windows; per window a full MHA block:
  q/k/v proj (fp32r matmuls), scores = qk^T + pos_bias + mask,
  softmax (free axis, ACT Exp with accumulated row sums),
  z = attn @ v, out = z @ Wp^T + bp.
"""
import os
import numpy as np

import concourse.bass as bass
import concourse.mybir as mybir
import concourse.tile as tile
from concourse import bacc
from concourse.bass_utils import run_bass_kernel_spmd
from concourse.masks import make_identity

B, W, S, E = 8, 64, 256, 512
H, D = 8, 64
SCALE = D ** -0.5
NCORES = 8
F32 = mybir.dt.float32
F32R = mybir.dt.float32r
AOp = mybir.AluOpType
AF = mybir.ActivationFunctionType


def _emit(nc, tc, ctx, n_w, d):
    """Emit the per-core program: n_w windows of MHA."""
    const = ctx.enter_context(tc.tile_pool(name="const", bufs=1))
    wstage = ctx.enter_context(tc.tile_pool(name="wstage", bufs=2))

    # --- one-time: weights (rounded to fp32r), biases, pos_bias, identity ---
    w_sb = {}
    for name in ("wq", "wk", "wv", "wp"):
        st = wstage.tile([128, 4, E], F32, tag="wst")
        nc.sync.dma_start(st[:], d[name].rearrange("(ic p) o -> p ic o", p=128))
        t = const.tile([128, 4, E], F32R, tag=name)
        nc.vector.tensor_copy(t[:], st[:])
        w_sb[name] = t

    bq_t = const.tile([128, 4, S], F32)
    nc.sync.dma_start(bq_t[:], d["bq"][:])
    bk_t = const.tile([128, 4, S], F32)
    nc.sync.dma_start(bk_t[:], d["bk"][:])
    bv_bc = const.tile([128, E], F32)
    nc.sync.dma_start(bv_bc[:], d["bv"][:])
    bp_bc = const.tile([128, E], F32)
    nc.sync.dma_start(bp_bc[:], d["bp"][:])

    pos_sb = const.tile([128, H, 2, S], F32)
    nc.sync.dma_start(pos_sb[:], d["pos"].rearrange("h (c p) j -> p h c j", p=128))

    ident = const.tile([128, 128], F32)
    make_identity(nc, ident[:])

    # --- pools for the per-window pipeline ---
    xnat_p = ctx.enter_context(tc.tile_pool(name="xnat", bufs=2))
    msk_p = ctx.enter_context(tc.tile_pool(name="msk", bufs=2))
    mpb_p = ctx.enter_context(tc.tile_pool(name="mpb", bufs=2))
    xt_p = ctx.enter_context(tc.tile_pool(name="xt", bufs=2))
    qkv_p = ctx.enter_context(tc.tile_pool(name="qkv", bufs=2))
    zt_p = ctx.enter_context(tc.tile_pool(name="zt", bufs=2))
    outs_p = ctx.enter_context(tc.tile_pool(name="outs", bufs=2))
    attn_p = ctx.enter_context(tc.tile_pool(name="attn", bufs=4))
    expt_p = ctx.enter_context(tc.tile_pool(name="expt", bufs=4))
    sums_p = ctx.enter_context(tc.tile_pool(name="sums", bufs=8))

    ps_pj = ctx.enter_context(tc.tile_pool(name="ps_pj", bufs=2, space="PSUM"))
    ps_sc = ctx.enter_context(tc.tile_pool(name="ps_sc", bufs=2, space="PSUM"))
    ps_tp = ctx.enter_context(tc.tile_pool(name="ps_tp", bufs=2, space="PSUM"))
    ps_z = ctx.enter_context(tc.tile_pool(name="ps_z", bufs=2, space="PSUM"))

    for w in range(n_w):
        # load x window [256, 512] as [128, (s-chunk, e)]
        xnat = xnat_p.tile([128, 2, E], F32, tag="xn")
        nc.sync.dma_start(xnat[:], d["x"][w].rearrange("(c p) e -> p c e", p=128))
        msk = msk_p.tile([128, 2, S], F32, tag="mk")
        nc.sync.dma_start(msk[:], d["mask"][w].rearrange("(c p) j -> p c j", p=128))

        # mask + pos_bias per head (gpsimd, sbuf only)
        mpb = mpb_p.tile([128, H, 2, S], F32, tag="mpb")
        for h in range(H):
            nc.gpsimd.tensor_tensor(mpb[:, h], msk[:], pos_sb[:, h], AOp.add)

        # xT [e, s] via PE transposes: [128 (e%128), (ec, s)]
        xT = xt_p.tile([128, 4, S], F32R, tag="xT")
        for ec in range(4):
            pt = ps_tp.tile([128, 2, 128], F32, tag="pt")
            for c in range(2):
                nc.tensor.transpose(pt[:, c], xnat[:, c, ec * 128:(ec + 1) * 128], ident[:])
            nc.vector.tensor_copy(xT[:, ec], pt[:])

        # projections: qT/kT [o, s] layout [128 (o%128), (oc, s)]
        qT = qkv_p.tile([128, 4, S], F32R, tag="qT")
        kT = qkv_p.tile([128, 4, S], F32R, tag="kT")
        for oc in range(4):
            pq = ps_pj.tile([128, S], F32, tag="pj")
            for ic in range(4):
                nc.tensor.matmul(pq[:], w_sb["wq"][:, ic, oc * 128:(oc + 1) * 128],
                                 xT[:, ic], start=(ic == 0), stop=(ic == 3))
            nc.vector.scalar_tensor_tensor(qT[:, oc], pq[:], 0.0, bq_t[:, oc],
                                           AOp.bypass, AOp.add)
            pk = ps_pj.tile([128, S], F32, tag="pj")
            for ic in range(4):
                nc.tensor.matmul(pk[:], w_sb["wk"][:, ic, oc * 128:(oc + 1) * 128],
                                 xT[:, ic], start=(ic == 0), stop=(ic == 3))
            nc.vector.scalar_tensor_tensor(kT[:, oc], pk[:], 0.0, bk_t[:, oc],
                                           AOp.bypass, AOp.add)

        # v natural [s, o] layout [128 (s%128), (sc, o)]
        vS = qkv_p.tile([128, 2, E], F32R, tag="vS")
        for sc in range(2):
            pv = ps_pj.tile([128, E], F32, tag="pj")
            for ic in range(4):
                nc.tensor.matmul(pv[:], xT[:, ic, sc * 128:(sc + 1) * 128],
                                 w_sb["wv"][:, ic], start=(ic == 0), stop=(ic == 3))
            nc.vector.scalar_tensor_tensor(vS[:, sc], pv[:], 0.0, bv_bc[:],
                                           AOp.bypass, AOp.add)

        # attention per head; zT [e, s] layout [128 (e%128), (hp, s)]
        zT = zt_p.tile([128, 4, S], F32R, tag="zT")
        for h in range(H):
            oc, prow = h // 2, (h % 2) * 64
            pts = [ps_tp.tile([128, 2, 128], F32, tag="pt", name=f"pt{h}_{j}") for j in range(2)]
            for icn in range(2):
                ps_s = ps_sc.tile([128, S], F32, tag="sc")
                nc.tensor.matmul(ps_s[:],
                                 qT[prow:prow + 64, oc, icn * 128:(icn + 1) * 128],
                                 kT[prow:prow + 64, oc], start=True, stop=True)
                ast = attn_p.tile([128, S], F32, tag="ast")
                nc.vector.scalar_tensor_tensor(ast[:], ps_s[:], 0.0, mpb[:, h, icn],
                                               AOp.bypass, AOp.add)
                aex = attn_p.tile([128, S], F32, tag="aex")
                sums = sums_p.tile([128, 1], F32, tag="sums")
                nc.scalar.activation(aex[:], ast[:], AF.Exp, accum_out=sums[:])
                rec = sums_p.tile([128, 1], F32, tag="rec")
                nc.vector.reciprocal(rec[:], sums[:])
                arn = attn_p.tile([128, S], F32, tag="arn")
                nc.scalar.activation(arn[:], aex[:], AF.Copy, scale=rec[:])
                # transpose the two 128-blocks of this row chunk
                for jc in range(2):
                    nc.tensor.transpose(pts[jc][:, icn],
                                        arn[:, jc * 128:(jc + 1) * 128], ident[:])
            expTs = []
            for jc in range(2):
                et = expt_p.tile([128, S], F32R, tag="expT", name=f"expT{h}_{jc}")
                nc.vector.tensor_copy(et[:], pts[jc][:])
                expTs.append(et)
            # z^T_h [d, s] accumulated over j chunks
            zh = ps_z.tile([64, S], F32, tag="zz", name=f"zh{h}")
            for jc in range(2):
                nc.tensor.matmul(zh[:], vS[:, jc, h * 64:(h + 1) * 64], expTs[jc][:],
                                 start=(jc == 0), stop=(jc == 1))
            nc.vector.tensor_copy(zT[prow:prow + 64, h // 2], zh[:])

        # output projection [s, o] natural + bias, then store
        out_sb = outs_p.tile([128, 2, E], F32, tag="osb")
        for sc in range(2):
            po = ps_pj.tile([128, E], F32, tag="pj")
            for ec in range(4):
                nc.tensor.matmul(po[:], zT[:, ec, sc * 128:(sc + 1) * 128],
                                 w_sb["wp"][:, ec], start=(ec == 0), stop=(ec == 3))
            nc.vector.scalar_tensor_tensor(out_sb[:, sc], po[:], 0.0, bp_bc[:],
                                           AOp.bypass, AOp.add)
        nc.sync.dma_start(d["out"][w].rearrange("(c p) e -> p c e", p=128), out_sb[:])


def _build(n_w):
    nc = bacc.Bacc("TRN2", target_bir_lowering=False, debug=False)
    d = {
        "x": nc.dram_tensor("x", [n_w, S, E], F32, kind="ExternalInput"),
        "mask": nc.dram_tensor("mask", [n_w, S, S], F32, kind="ExternalInput"),
        "pos": nc.dram_tensor("pos", [H, S, S], F32, kind="ExternalInput"),
        "wq": nc.dram_tensor("wq", [E, E], F32, kind="ExternalInput"),
        "wk": nc.dram_tensor("wk", [E, E], F32, kind="ExternalInput"),
        "wv": nc.dram_tensor("wv", [E, E], F32, kind="ExternalInput"),
        "wp": nc.dram_tensor("wp", [E, E], F32, kind="ExternalInput"),
        "bq": nc.dram_tensor("bq", [128, 4, S], F32, kind="ExternalInput"),
        "bk": nc.dram_tensor("bk", [128, 4, S], F32, kind="ExternalInput"),
        "bv": nc.dram_tensor("bv", [128, E], F32, kind="ExternalInput"),
        "bp": nc.dram_tensor("bp", [128, E], F32, kind="ExternalInput"),
        "out": nc.dram_tensor("out", [n_w, S, E], F32, kind="ExternalOutput"),
    }
    from contextlib import ExitStack
    with tile.TileContext(nc) as tc, ExitStack() as ctx:
        _emit(nc, tc, ctx, n_w, d)
    nc.compile()
    return nc


_NC_CACHE = {}


def _get_nc(n_w):
    if n_w not in _NC_CACHE:
        _NC_CACHE[n_w] = _build(n_w)
    return _NC_CACHE[n_w]


def _host_prep(mask, Wq, bq, Wk, bk, Wv, bv, Wp, bp, pos_bias):
    """Shared (replicated) input tensors, host-side layout prep."""
    f = np.float32
    wq_t = np.ascontiguousarray(Wq.T * SCALE, dtype=f)  # [in, out], SCALE folded
    wk_t = np.ascontiguousarray(Wk.T, dtype=f)
    wv_t = np.ascontiguousarray(Wv.T, dtype=f)
    wp_t = np.ascontiguousarray(Wp.T, dtype=f)
    bq_s = (bq * SCALE).astype(f)
    # bias tiles for qT/kT layout: [128 (o%128), oc, s] broadcast along s
    bq_t = np.ascontiguousarray(
        np.broadcast_to(bq_s.reshape(4, 128).T[:, :, None], (128, 4, S)))
    bk_t = np.ascontiguousarray(
        np.broadcast_to(np.asarray(bk, f).reshape(4, 128).T[:, :, None], (128, 4, S)))
    bv_bc = np.ascontiguousarray(np.broadcast_to(np.asarray(bv, f)[None, :], (128, E)))
    bp_bc = np.ascontiguousarray(np.broadcast_to(np.asarray(bp, f)[None, :], (128, E)))
    maskn = np.ascontiguousarray(np.asarray(mask, f)[0, :, 0])  # [W, S, S]
    pos = np.ascontiguousarray(np.asarray(pos_bias, f))
    return {
        "wq": wq_t, "wk": wk_t, "wv": wv_t, "wp": wp_t,
        "bq": bq_t, "bk": bk_t, "bv": bv_bc, "bp": bp_bc,
        "pos": pos, "_maskn": maskn,
    }


def kernel(x, mask, Wq, bq, Wk, bk, Wv, bv, Wp, bp, pos_bias, _trace=False):
    n_w = int(os.environ.get("KERNEL_NW", W))
    n_cores = NCORES
    x = np.asarray(x, np.float32)
    shared = _host_prep(mask, Wq, bq, Wk, bk, Wv, bv, Wp, bp, pos_bias)
    maskn = shared.pop("_maskn")[:n_w]

    in_maps = []
    for c in range(n_cores):
        m = dict(shared)
        m["mask"] = maskn
        m["x"] = np.ascontiguousarray(x[c % B, :n_w])
        in_maps.append(m)

    nc = _get_nc(n_w)
    res = run_bass_kernel_spmd(nc, in_maps, list(range(n_cores)), trace=_trace,
                               tmpdir=(os.environ.get("KERNEL_TRACE_DIR") if _trace else None))
    out = np.stack([res.results[c]["out"] for c in range(B)], axis=0)
    if _trace:
        kernel._last_exec_time_ns = res.exec_time_ns
        kernel._last_results = res
    return out
